# revision 52
# baseline (speedup 1.0000x reference)
"""Llama attention (B=2, S=2048, E=4096, H=32) on 8 trn2 NeuronCores.

Strategy (tensor-parallel over heads, 4 heads/core, all-bf16 datapath):
  - RoPE is position-independent here (cos/sin are [H, D/2]), so it is folded
    into wq/wk on the host; the 1/sqrt(D) scale is folded into wq too.
  - All matmul operands are bf16 (1 cycle/row on the PE, fast weight load),
    accumulation in fp32 PSUM.  The full E=4096 contraction of the Q/K/V
    projections accumulates in PSUM (32 chained matmuls per output tile); a
    single scalar-engine copy (with bf16 cast) evacuates each tile.
  - Scores are computed transposed (S^T = K^T-tile @ Q^T) and the attention
    output as O^T = V-tile @ P^T: zero on-device transposes.  exp() runs on
    the scalar engine over [128,1024] PSUM spans.  Softmax needs no
    max-subtraction (scores bounded ~ +-8; exp cannot overflow in fp32).
  - Softmax denominators on the PE: ones[128,128]^T @ esAB sums the exp'd
    scores over key partitions and broadcasts per-query sums to all
    partitions in one cheap matmul (the tile scheduler's sim models this
    accurately, unlike the gpsimd partition_all_reduce it replaced); then
    DVE reciprocal + multiply, deferred a few steps so the in-order DVE
    queue never parks on a cross-engine wait.
  - Attention (phase 2) and the output projection (phase 3) are merged into
    one interleaved PE stream: "py units" (one [128,512] column of Y) become
    ready one 512-query block after their queries finish attention and are
    paced ~1 per score/PV step, so the PE always has dependency-free work
    while the exp/softmax chain catches up.  wo streams through a 4-slot
    LRU ring in boustrophedon nE order (slice reuse at block boundaries).
  - The normalized attention output is written back in place over the dead
    Q block of its (head, query-block) chunk, so no separate OTT buffer is
    needed; the freed SBUF holds a prefetch of the next batch's first
    half-chunk of x, issued mid-stream before the drain floods the DMA
    queues, which makes the batch transition seamless.
  - Per-core output is a partial Y (row-sharded wo) written bf16 as
    contiguous [b, nE, m, 128, 512] unit blocks (large DMA descriptors);
    the host sums the 8 partials in fp32 and un-permutes once.
"""

import sys

sys.path.insert(0, "/opt/trn_rl_repo")

import numpy as np
import ml_dtypes

B, S, E, H = 2, 2048, 4096, 32
D = 128            # head dim
NCORES = 8
HL = H // NCORES   # heads per core = 4
W = HL * D         # per-core projection width = 512
T = B * S          # 4096 tokens
NKB = 32           # 128-row contraction tiles over E
CH1 = 256          # phase-1 token chunk
NCH1 = S // CH1    # 8 chunks per batch
NTT = S // 128     # 16 token/key tiles per batch

_CACHE = {}


def _build_nc():
    import concourse.bass as bass  # noqa: F401
    import concourse.mybir as mybir
    import concourse.tile as tile
    from concourse import bacc
    from concourse.bass_isa import ReduceOp

    fp32 = mybir.dt.float32
    bf16 = mybir.dt.bfloat16
    EXP = mybir.ActivationFunctionType.Exp

    nc = bacc.Bacc("TRN2", target_bir_lowering=False, debug=False)

    xh_d = nc.dram_tensor("xh", [T // CH1, 128, NKB, CH1], bf16, kind="ExternalInput")
    wqk_d = nc.dram_tensor("wqk", [128, NKB, 2 * W], bf16, kind="ExternalInput")
    wv_d = nc.dram_tensor("wv", [128, NKB, W], bf16, kind="ExternalInput")
    wo_d = nc.dram_tensor("wo", [128, HL, E], bf16, kind="ExternalInput")
    # per-unit contiguous blocks (fewer, larger DMA descriptors); the host
    # permutes [b, nE, m, row, col] -> [b, m*128+row, nE*512+col] after summing
    y_d = nc.dram_tensor("y", [B, 8, NTT, 128, 512], bf16, kind="ExternalOutput")

    with nc.allow_low_precision(reason="bf16 datapath; fp32 PSUM accumulation"), \
         tile.TileContext(nc) as tc:
        with tc.tile_pool(name="const", bufs=1) as constp, \
             tc.tile_pool(name="gw", bufs=1) as gwp, \
             tc.tile_pool(name="gwo", bufs=1) as wop, \
             tc.tile_pool(name="xpre", bufs=1) as xprep:
            zbias = constp.tile([128, 1], fp32, tag="zbias")
            nc.vector.memset(zbias[:], 0.0)
            ones = constp.tile([128, 128], bf16, tag="ones")
            nc.vector.memset(ones[:], 1.0)
            # pre-warm the ACT exp table during startup so the first real
            # exp in phase 2 doesn't pay the table load
            warm = constp.tile([128, 1], fp32, tag="warm")
            nc.scalar.activation(warm[:], zbias[:], EXP, bias=zbias[:, 0:1])

            wqk_t = gwp.tile([128, NKB, 2 * W], bf16, tag="wqk")
            wv_t = gwp.tile([128, NKB, W], bf16, tag="wv")

            # 4-slot LRU ring of 512-col wo slices, persistent across batches
            wo_state = {"slots": [None, None, None, None], "stamp": 0}
            xn_pre = {}   # batch -> prefetched first-half of its chunk 0

            for b in range(B):
                with tc.tile_pool(name=f"bt{b}", bufs=1) as btp:
                    # [*, 0:HL, sq, :]  = Q blocks; after chunk (h, sq) is
                    # done its Q block is dead and the normalized attention
                    # output for (h, sq) is written back IN PLACE -- the
                    # output projection reads its stationaries from here.
                    # [*, HL:2HL, :, :] = K, [sq, j] indexing key tile 4sq+j.
                    QKT = btp.tile([128, 2 * HL, 4, 512], bf16, tag="qkt",
                                   name="qkt")
                    V = btp.tile([128, NTT, W], bf16, tag="v", name="v")

                    # ---------------- phase 1: projections ----------------
                    with nc.named_scope(f"ph1b{b}"), \
                         tc.tile_pool(name=f"p1x{b}", bufs=2) as xpool, \
                         tc.tile_pool(name=f"p1qk{b}", bufs=1, space="PSUM") as psqk, \
                         tc.tile_pool(name=f"p1v{b}", bufs=2, space="PSUM") as psv:
                        for c in range(NCH1):
                            xn = xpool.tile([128, NKB, CH1], bf16, tag="xn")
                            xpre_t = None
                            if c == 0 and b in xn_pre:
                                # first half of chunk 0 was prefetched during
                                # the previous batch's merged phase
                                xpre_t = xn_pre.pop(b)
                                nc.sync.dma_start(xn[:, 16:NKB, :],
                                                  xh_d[b * NCH1, :, 16:NKB, :])
                            elif b == 0 and c == 0:
                                # fine-grained first loads, wv interleaved, so
                                # the PE starts early and V never stalls
                                for lo, hi in ((0, 1), (1, 2), (2, 4)):
                                    nc.sync.dma_start(xn[:, lo:hi, :],
                                                      xh_d[0, :, lo:hi, :])
                                    nc.sync.dma_start(wqk_t[:, lo:hi, :],
                                                      wqk_d[:, lo:hi, :])
                                nc.sync.dma_start(xn[:, 4:8, :],
                                                  xh_d[0, :, 4:8, :])
                                nc.sync.dma_start(wqk_t[:, 4:8, :],
                                                  wqk_d[:, 4:8, :])
                                nc.sync.dma_start(xn[:, 8:NKB, :],
                                                  xh_d[0, :, 8:NKB, :])
                                nc.sync.dma_start(wqk_t[:, 8:16, :],
                                                  wqk_d[:, 8:16, :])
                                nc.sync.dma_start(wv_t[:, 0:8, :],
                                                  wv_d[:, 0:8, :])
                                nc.sync.dma_start(wqk_t[:, 16:24, :],
                                                  wqk_d[:, 16:24, :])
                                nc.sync.dma_start(wv_t[:, 8:16, :],
                                                  wv_d[:, 8:16, :])
                                nc.sync.dma_start(wqk_t[:, 24:32, :],
                                                  wqk_d[:, 24:32, :])
                                nc.sync.dma_start(wv_t[:, 16:32, :],
                                                  wv_d[:, 16:32, :])
                            else:
                                nc.sync.dma_start(xn[:], xh_d[b * NCH1 + c])
                            pqk = psqk.tile([128, 2 * HL, CH1], fp32, tag="pqk",
                                            name="pqk")
                            pv = psv.tile([128, 2, W], fp32, tag="pv", name="pv")

                            def xs(kb):
                                if xpre_t is not None and kb < 16:
                                    return xpre_t[:, kb, :]
                                return xn[:, kb, :]

                            def emit_v():
                                for kb in range(NKB):
                                    xk = xs(kb)
                                    for ts in range(2):
                                        nc.tensor.matmul(
                                            pv[:, ts, :],
                                            xk[:, ts * 128:(ts + 1) * 128],
                                            wv_t[:, kb, :],
                                            start=(kb == 0), stop=(kb == NKB - 1),
                                        )

                            def emit_qk():
                                for kb in range(NKB):
                                    xk = xs(kb)
                                    for t in range(2 * HL):
                                        # two [128,256] tiles share one PSUM
                                        # bank; start clears the WHOLE bank's
                                        # has_written bits, so only the first
                                        # matmul touching each bank may carry
                                        # start=True.
                                        nc.tensor.matmul(
                                            pqk[:, t, :],
                                            wqk_t[:, kb, t * 128:(t + 1) * 128],
                                            xk[:],
                                            start=(kb == 0 and t % 2 == 0),
                                            stop=(kb == NKB - 1 and t % 2 == 1),
                                        )

                            if c == 0:
                                emit_qk()
                                emit_v()
                                nc.vector.tensor_copy(
                                    V[:, 2 * c:2 * c + 2, :], pv[:])
                            else:
                                # pv copy between the V and QK matmul groups:
                                # it completes under the QK cover, so the
                                # merged phase's first PV consumers never
                                # wait on the last chunk's V evacuation
                                emit_v()
                                nc.vector.tensor_copy(
                                    V[:, 2 * c:2 * c + 2, :], pv[:])
                                emit_qk()
                            if c < NCH1 - 1:
                                nc.scalar.copy(
                                    QKT[:, :, c // 2,
                                        (c % 2) * CH1:(c % 2 + 1) * CH1],
                                    pqk[:])
                            else:
                                # split the last chunk's copy: the K half is
                                # needed by the attention stream's group-7
                                # score matmuls almost immediately, the Q
                                # half only by sq=3 chunks much later
                                nc.scalar.copy(
                                    QKT[:, HL:2 * HL, c // 2,
                                        (c % 2) * CH1:(c % 2 + 1) * CH1],
                                    pqk[:, HL:2 * HL, :])
                                nc.vector.tensor_copy(
                                    QKT[:, 0:HL, c // 2,
                                        (c % 2) * CH1:(c % 2 + 1) * CH1],
                                    pqk[:, 0:HL, :])

                    # ------- phase 2+3 merged: attention + output proj -------
                    # chunk = (head h, 512-query block sq).  Output-projection
                    # "py units" (one [128,512] column of Y per unit) become
                    # ready one sq-block after their queries finish attention;
                    # they are interleaved one-per-step into the score/PV
                    # stream so the PE always has dependency-free work while
                    # the exp/softmax-tail chain catches up.  wo streams
                    # through a 3-slot ring, one 512-col slice per unit group.
                    with nc.named_scope(f"ph23b{b}"), \
                         tc.tile_pool(name=f"a2e{b}", bufs=6) as ep, \
                         tc.tile_pool(name=f"a2s{b}", bufs=2) as esp, \
                         tc.tile_pool(name=f"a2r{b}", bufs=2) as rcp, \
                         tc.tile_pool(name=f"p3y{b}", bufs=6) as yp3, \
                         tc.tile_pool(name=f"a2ps{b}", bufs=2, space="PSUM") as psP:
                        state = {}
                        pend_tail = []   # (due_k, ci)
                        units_done = 0
                        units_ready = 0
                        drain_n = 0

                        # unit order: blocks msq=0..3, boustrophedon nE so
                        # the LRU wo ring reuses 3 slices at every block (and
                        # batch) boundary: 8+5+5+... loads instead of 8/block
                        unit_list = []
                        group_slices = []
                        for msq in range(4):
                            fwd = (b * 4 + msq) % 2 == 0
                            nEs = range(8) if fwd else range(7, -1, -1)
                            for nE in nEs:
                                group_slices.append(nE)
                                for j in range(4):
                                    unit_list.append((4 * msq + j, nE))
                        group_ap = [None] * 32
                        wo_issued = 0

                        def ensure_wo(upto):
                            nonlocal wo_issued
                            slots = wo_state["slots"]
                            while wo_issued < min(upto, 32):
                                sl = group_slices[wo_issued]
                                hit = next((s for s in slots
                                            if s and s[0] == sl), None)
                                wo_state["stamp"] += 1
                                if hit is None:
                                    idx = min(range(len(slots)), key=lambda i:
                                              slots[i][2] if slots[i] else -1)
                                    ap = wop.tile([128, HL, 512], bf16,
                                                  tag=f"wo{idx}",
                                                  name=f"wo_{b}_{wo_issued}")
                                    nc.sync.dma_start(
                                        ap[:],
                                        wo_d[:, :, sl * 512:(sl + 1) * 512])
                                    slots[idx] = [sl, ap, wo_state["stamp"]]
                                    hit = slots[idx]
                                else:
                                    hit[2] = wo_state["stamp"]
                                group_ap[wo_issued] = hit[1]
                                wo_issued += 1

                        def emit_unit(drain=False):
                            nonlocal units_done
                            if units_done >= units_ready:
                                return
                            g = units_done // 4
                            ensure_wo(g + 3)
                            m, nE = unit_list[units_done]
                            wo_t = group_ap[g]
                            py = psP.tile([128, 512], fp32, tag="py", name="py")
                            for kd in range(HL):
                                nc.tensor.matmul(
                                    py[:],
                                    QKT[:, kd, m // 4,
                                        (m % 4) * 128:(m % 4 + 1) * 128],
                                    wo_t[:, kd, :],
                                    start=(kd == 0), stop=(kd == HL - 1),
                                )
                            yt = yp3.tile([128, 512], bf16, tag="yt")
                            # early drain: ACT only (DVE still runs the last
                            # tail muls); later alternate so neither engine's
                            # copy latency gates the py ring
                            nonlocal drain_n
                            if drain:
                                drain_n += 1
                            if (drain and drain_n <= 4) or units_done % 2 == 0:
                                nc.scalar.copy(yt[:], py[:])
                            else:
                                nc.vector.tensor_copy(yt[:], py[:])
                            nc.sync.dma_start(y_d[b, nE, m], yt[:])
                            units_done += 1

                        def emit_tail_denom(ci):
                            # per-query softmax denominator via the PE:
                            # ones^T @ esAB sums over the 128 key partitions
                            # and broadcasts the result to every partition.
                            # Unlike gpsimd partition_all_reduce this is
                            # cheap (2x512 cycles), modeled accurately by the
                            # tile scheduler's sim, and keeps the in-order
                            # DVE queue free of long cross-engine waits.
                            po, esAB, _ = state[ci]
                            denomP = psP.tile([128, 512], fp32, tag="py",
                                              name="denomP")
                            for j in range(2):
                                nc.tensor.matmul(
                                    denomP[:], ones[:], esAB[:, j, :],
                                    start=(j == 0), stop=(j == 1),
                                )
                            state[ci][2] = denomP

                        def emit_tail_end(ci):
                            h, sq = ci
                            po, esAB, denomP = state.pop(ci)
                            rr = rcp.tile([128, 512], fp32, tag="rr")
                            nc.vector.reciprocal_approx_fast(rr[:], denomP[:])
                            # normalized output written back over the (dead)
                            # Q block of (h, sq)
                            nc.vector.tensor_mul(
                                QKT[:, h, sq, :], po[:], rr[:])
                            if h == HL - 1:
                                # all heads of sq done -> its 32 units ready
                                nonlocal units_ready
                                units_ready += 32

                        def consume(k, ci, g, eS):
                            h, sq = ci
                            po, esAB, _ = state[ci]
                            for j in range(2):
                                sk = 2 * g + j
                                nc.tensor.matmul(
                                    po[:],
                                    V[:, sk, h * 128:(h + 1) * 128],
                                    eS[:, j, :],
                                    start=(sk == 0), stop=(sk == 15),
                                )
                            if g == 0:
                                nc.vector.tensor_copy(esAB[:], eS[:])
                            else:
                                nc.vector.tensor_add(esAB[:], esAB[:], eS[:])
                            if g == 7:
                                # denom matmul 3 steps later (esAB add surely
                                # done), recip+mul one step after that
                                pend_tail.append((k + 2, 0, ci))
                                pend_tail.append((k + 3, 1, ci))

                        # prime the first two wo slices
                        ensure_wo(2)

                        # flat group stream, PV consumption lagging LAG groups
                        # behind the pS/exp production so the PE never waits
                        # on a fresh exp (the in-order queue always has pS
                        # work between an exp and its PV consumer)
                        LAG = 3
                        chunks = [(h, sq) for sq in range(4) for h in range(HL)]
                        stream = [(ci, g) for ci in chunks for g in range(8)]
                        fifo = []
                        for k, (ci, g) in enumerate(stream):
                            h, sq = ci
                            q0 = sq * 512
                            if g == 0:
                                po = psP.tile([128, 512], fp32, tag="po",
                                              name="po")
                                esAB = esp.tile([128, 2, 512], bf16,
                                                tag="esAB")
                                state[ci] = [po, esAB, None]
                            pS = psP.tile([128, 2, 512], fp32, tag="pS",
                                          name="pS")
                            for j in range(2):
                                sk = 2 * g + j
                                nc.tensor.matmul(
                                    pS[:, j, :],
                                    QKT[:, HL + h, sk // 4,
                                        (sk % 4) * 128:(sk % 4 + 1) * 128],
                                    QKT[:, h, sq, :],
                                    start=True, stop=True,
                                )
                            eS = ep.tile([128, 2, 512], bf16, tag="eS")
                            nc.scalar.activation(eS[:], pS[:], EXP,
                                                 bias=zbias[:, 0:1])
                            fifo.append((ci, g, eS))
                            # deeper lag for the first steps: the batch's
                            # first exp pays ACT wake-up latency
                            lag = 5 if k < 16 else LAG
                            while len(fifo) > lag:
                                cci, cg, ceS = fifo.pop(0)
                                consume(k, cci, cg, ceS)
                            while pend_tail and pend_tail[0][0] <= k:
                                _, kind, tci = pend_tail.pop(0)
                                if kind == 0:
                                    emit_tail_denom(tci)
                                else:
                                    emit_tail_end(tci)
                            emit_unit()
                            # catch up if behind the 1-unit-per-step schedule
                            # (units arrive ~4 steps into each block)
                            while units_done < min(max(0, k - 36),
                                                   units_ready):
                                emit_unit()
                            if k == 64 and b + 1 < B:
                                # prefetch the next batch's first half-chunk
                                # now, ahead of the drain's DMA burst, so
                                # ph1(b+1) starts without waiting on queues
                                xpt = xprep.tile([128, 16, CH1], bf16,
                                                 tag="xpre", name=f"xpre{b+1}")
                                nc.sync.dma_start(
                                    xpt[:], xh_d[(b + 1) * NCH1, :, 0:16, :])
                                xn_pre[b + 1] = xpt
                        k = len(stream)
                        while fifo:
                            cci, cg, ceS = fifo.pop(0)
                            consume(k, cci, cg, ceS)
                            k += 1
                        # leftover ready units: dependency-free PE cover
                        # while the last chunk's tail chain (DVE add ->
                        # denom matmul -> recip -> mul) resolves
                        for _ in range(min(6, units_ready - units_done)):
                            emit_unit()
                        while pend_tail:
                            _, kind, tci = pend_tail.pop(0)
                            if kind == 0:
                                emit_tail_denom(tci)
                            else:
                                emit_tail_end(tci)
                        while units_done < len(unit_list):
                            emit_unit(drain=True)

    nc.compile()
    return nc


def _prep_inputs(x, freqs_cos, freqs_sin, wq, wk, wv, wo):
    x = np.asarray(x, np.float32)
    c = np.asarray(freqs_cos, np.float32)
    s = np.asarray(freqs_sin, np.float32)
    wq = np.asarray(wq, np.float32)
    wk = np.asarray(wk, np.float32)
    wv = np.asarray(wv, np.float32)
    wo = np.asarray(wo, np.float32)
    bf = ml_dtypes.bfloat16

    xT = x.reshape(T, E).T.astype(bf)
    xh = np.ascontiguousarray(
        xT.reshape(NKB, 128, T // CH1, CH1).transpose(2, 1, 0, 3))

    def fold(w):
        wr = w.reshape(H, D // 2, 2, E)
        w0, w1 = wr[:, :, 0], wr[:, :, 1]
        r0 = c[:, :, None] * w0 - s[:, :, None] * w1
        r1 = s[:, :, None] * w0 + c[:, :, None] * w1
        return np.stack([r0, r1], axis=2).reshape(E, E)

    wq_r = fold(wq) * np.float32(D ** -0.5)
    wk_r = fold(wk)

    in_maps = []
    for cix in range(NCORES):
        sl = slice(cix * W, (cix + 1) * W)
        qk = np.concatenate([wq_r[sl].T, wk_r[sl].T], axis=1)   # [E, 2W]
        wqkh = np.ascontiguousarray(
            qk.astype(bf).reshape(NKB, 128, 2 * W).transpose(1, 0, 2))
        wvh = np.ascontiguousarray(
            wv[sl].T.astype(bf).reshape(NKB, 128, W).transpose(1, 0, 2))
        woh = np.ascontiguousarray(
            wo[:, sl].T.astype(bf).reshape(HL, 128, E).transpose(1, 0, 2))
        in_maps.append({"xh": xh, "wqk": wqkh, "wv": wvh, "wo": woh})
    return in_maps


def run(x, freqs_cos, freqs_sin, wq, wk, wv, wo, trace=False, tmpdir=None):
    from concourse.bass_utils import run_bass_kernel_spmd

    if "nc" not in _CACHE:
        _CACHE["nc"] = _build_nc()
    nc = _CACHE["nc"]
    in_maps = _prep_inputs(x, freqs_cos, freqs_sin, wq, wk, wv, wo)
    res = run_bass_kernel_spmd(
        nc, in_maps, list(range(NCORES)), trace=trace, tmpdir=tmpdir
    )
    y = np.asarray(res.results[0]["y"], np.float32)
    for r in res.results[1:]:
        y = y + np.asarray(r["y"], np.float32)
    # [b, nE, m, row, col] -> [b, m*128+row, nE*512+col]
    y = y.transpose(0, 2, 3, 1, 4).reshape(B, S, E)
    return y, res


def kernel(x, start_pos=0, freqs_cos=None, freqs_sin=None,
           wq=None, wk=None, wv=None, wo=None):
    y, _ = run(x, freqs_cos, freqs_sin, wq, wk, wv, wo)
    return y



# revision 53
# speedup vs baseline: 1.0057x; 1.0057x over previous
"""Llama attention (B=2, S=2048, E=4096, H=32) on 8 trn2 NeuronCores.

Strategy (tensor-parallel over heads, 4 heads/core, all-bf16 datapath):
  - RoPE is position-independent here (cos/sin are [H, D/2]), so it is folded
    into wq/wk on the host; the 1/sqrt(D) scale is folded into wq too.
  - All matmul operands are bf16 (1 cycle/row on the PE, fast weight load),
    accumulation in fp32 PSUM.  The full E=4096 contraction of the Q/K/V
    projections accumulates in PSUM (32 chained matmuls per output tile); a
    single scalar-engine copy (with bf16 cast) evacuates each tile.
  - Scores are computed transposed (S^T = K^T-tile @ Q^T) and the attention
    output as O^T = V-tile @ P^T: zero on-device transposes.  exp() runs on
    the scalar engine over [128,1024] PSUM spans.  Softmax needs no
    max-subtraction (scores bounded ~ +-8; exp cannot overflow in fp32).
  - Softmax denominators on the PE: ones[128,128]^T @ esAB sums the exp'd
    scores over key partitions and broadcasts per-query sums to all
    partitions in one cheap matmul (the tile scheduler's sim models this
    accurately, unlike the gpsimd partition_all_reduce it replaced); then
    DVE reciprocal + multiply, deferred a few steps so the in-order DVE
    queue never parks on a cross-engine wait.
  - Attention (phase 2) and the output projection (phase 3) are merged into
    one interleaved PE stream: "py units" (one [128,512] column of Y) become
    ready one 512-query block after their queries finish attention and are
    paced ~1 per score/PV step, so the PE always has dependency-free work
    while the exp/softmax chain catches up.  wo streams through a 4-slot
    LRU ring in boustrophedon nE order (slice reuse at block boundaries).
  - The normalized attention output is written back in place over the dead
    Q block of its (head, query-block) chunk, so no separate OTT buffer is
    needed; the freed SBUF holds a prefetch of the next batch's first
    half-chunk of x, issued mid-stream before the drain floods the DMA
    queues, which makes the batch transition seamless.
  - Per-core output is a partial Y (row-sharded wo) written bf16 as
    contiguous [b, nE, m, 128, 512] unit blocks (large DMA descriptors);
    the host sums the 8 partials in fp32 and un-permutes once.
"""

import sys

sys.path.insert(0, "/opt/trn_rl_repo")

import numpy as np
import ml_dtypes

B, S, E, H = 2, 2048, 4096, 32
D = 128            # head dim
NCORES = 8
HL = H // NCORES   # heads per core = 4
W = HL * D         # per-core projection width = 512
T = B * S          # 4096 tokens
NKB = 32           # 128-row contraction tiles over E
CH1 = 256          # phase-1 token chunk
NCH1 = S // CH1    # 8 chunks per batch
NTT = S // 128     # 16 token/key tiles per batch

_CACHE = {}


def _build_nc():
    import concourse.bass as bass  # noqa: F401
    import concourse.mybir as mybir
    import concourse.tile as tile
    from concourse import bacc
    from concourse.bass_isa import ReduceOp

    fp32 = mybir.dt.float32
    bf16 = mybir.dt.bfloat16
    EXP = mybir.ActivationFunctionType.Exp

    nc = bacc.Bacc("TRN2", target_bir_lowering=False, debug=False)

    xh_d = nc.dram_tensor("xh", [T // CH1, 128, NKB, CH1], bf16, kind="ExternalInput")
    wqk_d = nc.dram_tensor("wqk", [128, NKB, 2 * W], bf16, kind="ExternalInput")
    wv_d = nc.dram_tensor("wv", [128, NKB, W], bf16, kind="ExternalInput")
    wo_d = nc.dram_tensor("wo", [128, HL, E], bf16, kind="ExternalInput")
    # per-unit contiguous blocks (fewer, larger DMA descriptors); the host
    # permutes [b, nE, m, row, col] -> [b, m*128+row, nE*512+col] after summing
    y_d = nc.dram_tensor("y", [B, 8, NTT, 128, 512], bf16, kind="ExternalOutput")

    with nc.allow_low_precision(reason="bf16 datapath; fp32 PSUM accumulation"), \
         tile.TileContext(nc) as tc:
        with tc.tile_pool(name="const", bufs=1) as constp, \
             tc.tile_pool(name="gw", bufs=1) as gwp, \
             tc.tile_pool(name="gwo", bufs=1) as wop, \
             tc.tile_pool(name="xpre", bufs=1) as xprep:
            zbias = constp.tile([128, 1], fp32, tag="zbias")
            nc.vector.memset(zbias[:], 0.0)
            ones = constp.tile([128, 128], bf16, tag="ones")
            nc.vector.memset(ones[:], 1.0)
            # pre-warm the ACT exp table during startup so the first real
            # exp in phase 2 doesn't pay the table load
            warm = constp.tile([128, 1], fp32, tag="warm")
            nc.scalar.activation(warm[:], zbias[:], EXP, bias=zbias[:, 0:1])

            wqk_t = gwp.tile([128, NKB, 2 * W], bf16, tag="wqk")
            wv_t = gwp.tile([128, NKB, W], bf16, tag="wv")

            # 4-slot LRU ring of 512-col wo slices, persistent across batches
            wo_state = {"slots": [None, None, None, None], "stamp": 0}
            xn_pre = {}   # batch -> prefetched first-half of its chunk 0

            for b in range(B):
                with tc.tile_pool(name=f"bt{b}", bufs=1) as btp:
                    # [*, 0:HL, sq, :]  = Q blocks; after chunk (h, sq) is
                    # done its Q block is dead and the normalized attention
                    # output for (h, sq) is written back IN PLACE -- the
                    # output projection reads its stationaries from here.
                    # [*, HL:2HL, :, :] = K, [sq, j] indexing key tile 4sq+j.
                    QKT = btp.tile([128, 2 * HL, 4, 512], bf16, tag="qkt",
                                   name="qkt")
                    V = btp.tile([128, NTT, W], bf16, tag="v", name="v")

                    # ---------------- phase 1: projections ----------------
                    with nc.named_scope(f"ph1b{b}"), \
                         tc.tile_pool(name=f"p1x{b}", bufs=2) as xpool, \
                         tc.tile_pool(name=f"p1qk{b}", bufs=1, space="PSUM") as psqk, \
                         tc.tile_pool(name=f"p1v{b}", bufs=2, space="PSUM") as psv:
                        for c in range(NCH1):
                            xn = xpool.tile([128, NKB, CH1], bf16, tag="xn")
                            xpre_t = None
                            if c == 0 and b in xn_pre:
                                # first half of chunk 0 was prefetched during
                                # the previous batch's merged phase
                                xpre_t = xn_pre.pop(b)
                                nc.sync.dma_start(xn[:, 16:NKB, :],
                                                  xh_d[b * NCH1, :, 16:NKB, :])
                            elif b == 0 and c == 0:
                                # fine-grained first loads, wv interleaved, so
                                # the PE starts early and V never stalls
                                for lo, hi in ((0, 1), (1, 2), (2, 4)):
                                    nc.sync.dma_start(xn[:, lo:hi, :],
                                                      xh_d[0, :, lo:hi, :])
                                    nc.sync.dma_start(wqk_t[:, lo:hi, :],
                                                      wqk_d[:, lo:hi, :])
                                nc.sync.dma_start(xn[:, 4:8, :],
                                                  xh_d[0, :, 4:8, :])
                                nc.sync.dma_start(wqk_t[:, 4:8, :],
                                                  wqk_d[:, 4:8, :])
                                nc.sync.dma_start(xn[:, 8:NKB, :],
                                                  xh_d[0, :, 8:NKB, :])
                                nc.sync.dma_start(wqk_t[:, 8:16, :],
                                                  wqk_d[:, 8:16, :])
                                nc.sync.dma_start(wv_t[:, 0:8, :],
                                                  wv_d[:, 0:8, :])
                                nc.sync.dma_start(wqk_t[:, 16:24, :],
                                                  wqk_d[:, 16:24, :])
                                nc.sync.dma_start(wv_t[:, 8:16, :],
                                                  wv_d[:, 8:16, :])
                                nc.sync.dma_start(wqk_t[:, 24:32, :],
                                                  wqk_d[:, 24:32, :])
                                nc.sync.dma_start(wv_t[:, 16:32, :],
                                                  wv_d[:, 16:32, :])
                            else:
                                nc.sync.dma_start(xn[:], xh_d[b * NCH1 + c])
                            pqk = psqk.tile([128, 2 * HL, CH1], fp32, tag="pqk",
                                            name="pqk")
                            pv = psv.tile([128, 2, W], fp32, tag="pv", name="pv")

                            def xs(kb):
                                if xpre_t is not None and kb < 16:
                                    return xpre_t[:, kb, :]
                                return xn[:, kb, :]

                            def emit_v():
                                for kb in range(NKB):
                                    xk = xs(kb)
                                    for ts in range(2):
                                        nc.tensor.matmul(
                                            pv[:, ts, :],
                                            xk[:, ts * 128:(ts + 1) * 128],
                                            wv_t[:, kb, :],
                                            start=(kb == 0), stop=(kb == NKB - 1),
                                        )

                            def emit_qk():
                                for kb in range(NKB):
                                    xk = xs(kb)
                                    for t in range(2 * HL):
                                        # two [128,256] tiles share one PSUM
                                        # bank; start clears the WHOLE bank's
                                        # has_written bits, so only the first
                                        # matmul touching each bank may carry
                                        # start=True.
                                        nc.tensor.matmul(
                                            pqk[:, t, :],
                                            wqk_t[:, kb, t * 128:(t + 1) * 128],
                                            xk[:],
                                            start=(kb == 0 and t % 2 == 0),
                                            stop=(kb == NKB - 1 and t % 2 == 1),
                                        )

                            if c == 0:
                                emit_qk()
                                emit_v()
                                nc.vector.tensor_copy(
                                    V[:, 2 * c:2 * c + 2, :], pv[:])
                            else:
                                # pv copy between the V and QK matmul groups:
                                # it completes under the QK cover, so the
                                # merged phase's first PV consumers never
                                # wait on the last chunk's V evacuation
                                emit_v()
                                nc.vector.tensor_copy(
                                    V[:, 2 * c:2 * c + 2, :], pv[:])
                                emit_qk()
                            if c < NCH1 - 1:
                                nc.scalar.copy(
                                    QKT[:, :, c // 2,
                                        (c % 2) * CH1:(c % 2 + 1) * CH1],
                                    pqk[:])
                            else:
                                # split the last chunk's copy: the K half is
                                # needed by the attention stream's group-7
                                # score matmuls almost immediately, the Q
                                # half only by sq=3 chunks much later
                                nc.scalar.copy(
                                    QKT[:, HL:2 * HL, c // 2,
                                        (c % 2) * CH1:(c % 2 + 1) * CH1],
                                    pqk[:, HL:2 * HL, :])
                                nc.vector.tensor_copy(
                                    QKT[:, 0:HL, c // 2,
                                        (c % 2) * CH1:(c % 2 + 1) * CH1],
                                    pqk[:, 0:HL, :])

                    # ------- phase 2+3 merged: attention + output proj -------
                    # chunk = (head h, 512-query block sq).  Output-projection
                    # "py units" (one [128,512] column of Y per unit) become
                    # ready one sq-block after their queries finish attention;
                    # they are interleaved one-per-step into the score/PV
                    # stream so the PE always has dependency-free work while
                    # the exp/softmax-tail chain catches up.  wo streams
                    # through a 3-slot ring, one 512-col slice per unit group.
                    with nc.named_scope(f"ph23b{b}"), \
                         tc.tile_pool(name=f"a2e{b}", bufs=6) as ep, \
                         tc.tile_pool(name=f"a2s{b}", bufs=2) as esp, \
                         tc.tile_pool(name=f"a2r{b}", bufs=2) as rcp, \
                         tc.tile_pool(name=f"p3y{b}", bufs=6) as yp3, \
                         tc.tile_pool(name=f"a2ps{b}", bufs=2, space="PSUM") as psP:
                        state = {}
                        pend_tail = []   # (due_k, ci)
                        units_done = 0
                        units_ready = 0
                        drain_n = 0

                        # unit order: blocks msq=0..3, boustrophedon nE so
                        # the LRU wo ring reuses 3 slices at every block (and
                        # batch) boundary: 8+5+5+... loads instead of 8/block
                        unit_list = []
                        group_slices = []
                        for msq in range(4):
                            fwd = (b * 4 + msq) % 2 == 0
                            nEs = range(8) if fwd else range(7, -1, -1)
                            for nE in nEs:
                                group_slices.append(nE)
                                for j in range(4):
                                    unit_list.append((4 * msq + j, nE))
                        group_ap = [None] * 32
                        wo_issued = 0

                        def ensure_wo(upto):
                            nonlocal wo_issued
                            slots = wo_state["slots"]
                            while wo_issued < min(upto, 32):
                                sl = group_slices[wo_issued]
                                hit = next((s for s in slots
                                            if s and s[0] == sl), None)
                                wo_state["stamp"] += 1
                                if hit is None:
                                    idx = min(range(len(slots)), key=lambda i:
                                              slots[i][2] if slots[i] else -1)
                                    ap = wop.tile([128, HL, 512], bf16,
                                                  tag=f"wo{idx}",
                                                  name=f"wo_{b}_{wo_issued}")
                                    nc.sync.dma_start(
                                        ap[:],
                                        wo_d[:, :, sl * 512:(sl + 1) * 512])
                                    slots[idx] = [sl, ap, wo_state["stamp"]]
                                    hit = slots[idx]
                                else:
                                    hit[2] = wo_state["stamp"]
                                group_ap[wo_issued] = hit[1]
                                wo_issued += 1

                        def emit_unit(drain=False):
                            nonlocal units_done
                            if units_done >= units_ready:
                                return
                            g = units_done // 4
                            ensure_wo(g + 4)
                            m, nE = unit_list[units_done]
                            wo_t = group_ap[g]
                            py = psP.tile([128, 512], fp32, tag="py", name="py")
                            for kd in range(HL):
                                nc.tensor.matmul(
                                    py[:],
                                    QKT[:, kd, m // 4,
                                        (m % 4) * 128:(m % 4 + 1) * 128],
                                    wo_t[:, kd, :],
                                    start=(kd == 0), stop=(kd == HL - 1),
                                )
                            yt = yp3.tile([128, 512], bf16, tag="yt")
                            # early drain: ACT only (DVE still runs the last
                            # tail muls); later alternate so neither engine's
                            # copy latency gates the py ring
                            nonlocal drain_n
                            if drain:
                                drain_n += 1
                            if (drain and drain_n <= 4) or units_done % 2 == 0:
                                nc.scalar.copy(yt[:], py[:])
                            else:
                                nc.vector.tensor_copy(yt[:], py[:])
                            nc.sync.dma_start(y_d[b, nE, m], yt[:])
                            units_done += 1

                        def emit_tail_denom(ci):
                            # per-query softmax denominator via the PE:
                            # ones^T @ esAB sums over the 128 key partitions
                            # and broadcasts the result to every partition.
                            # Unlike gpsimd partition_all_reduce this is
                            # cheap (2x512 cycles), modeled accurately by the
                            # tile scheduler's sim, and keeps the in-order
                            # DVE queue free of long cross-engine waits.
                            po, esAB, _ = state[ci]
                            denomP = psP.tile([128, 512], fp32, tag="py",
                                              name="denomP")
                            for j in range(2):
                                nc.tensor.matmul(
                                    denomP[:], ones[:], esAB[:, j, :],
                                    start=(j == 0), stop=(j == 1),
                                )
                            state[ci][2] = denomP

                        def emit_tail_end(ci):
                            h, sq = ci
                            po, esAB, denomP = state.pop(ci)
                            rr = rcp.tile([128, 512], fp32, tag="rr")
                            nc.vector.reciprocal_approx_fast(rr[:], denomP[:])
                            # normalized output written back over the (dead)
                            # Q block of (h, sq)
                            nc.vector.tensor_mul(
                                QKT[:, h, sq, :], po[:], rr[:])
                            if h == HL - 1:
                                # all heads of sq done -> its 32 units ready
                                nonlocal units_ready
                                units_ready += 32

                        def consume(k, ci, g, eS):
                            h, sq = ci
                            po, esAB, _ = state[ci]
                            for j in range(2):
                                sk = 2 * g + j
                                nc.tensor.matmul(
                                    po[:],
                                    V[:, sk, h * 128:(h + 1) * 128],
                                    eS[:, j, :],
                                    start=(sk == 0), stop=(sk == 15),
                                )
                            if g == 0:
                                nc.vector.tensor_copy(esAB[:], eS[:])
                            else:
                                nc.vector.tensor_add(esAB[:], esAB[:], eS[:])
                            if g == 7:
                                # denom matmul 3 steps later (esAB add surely
                                # done), recip+mul one step after that
                                pend_tail.append((k + 2, 0, ci))
                                pend_tail.append((k + 3, 1, ci))

                        # prime the first two wo slices
                        ensure_wo(2)

                        # flat group stream, PV consumption lagging LAG groups
                        # behind the pS/exp production so the PE never waits
                        # on a fresh exp (the in-order queue always has pS
                        # work between an exp and its PV consumer)
                        LAG = 3
                        chunks = [(h, sq) for sq in range(4) for h in range(HL)]
                        stream = [(ci, g) for ci in chunks for g in range(8)]
                        fifo = []
                        for k, (ci, g) in enumerate(stream):
                            h, sq = ci
                            q0 = sq * 512
                            if g == 0:
                                po = psP.tile([128, 512], fp32, tag="po",
                                              name="po")
                                esAB = esp.tile([128, 2, 512], bf16,
                                                tag="esAB")
                                state[ci] = [po, esAB, None]
                            pS = psP.tile([128, 2, 512], fp32, tag="pS",
                                          name="pS")
                            for j in range(2):
                                sk = 2 * g + j
                                nc.tensor.matmul(
                                    pS[:, j, :],
                                    QKT[:, HL + h, sk // 4,
                                        (sk % 4) * 128:(sk % 4 + 1) * 128],
                                    QKT[:, h, sq, :],
                                    start=True, stop=True,
                                )
                            eS = ep.tile([128, 2, 512], bf16, tag="eS")
                            nc.scalar.activation(eS[:], pS[:], EXP,
                                                 bias=zbias[:, 0:1])
                            fifo.append((ci, g, eS))
                            # deeper lag for the first steps: the batch's
                            # first exp pays ACT wake-up latency
                            lag = 5 if k < 16 else LAG
                            while len(fifo) > lag:
                                cci, cg, ceS = fifo.pop(0)
                                consume(k, cci, cg, ceS)
                            while pend_tail and pend_tail[0][0] <= k:
                                _, kind, tci = pend_tail.pop(0)
                                if kind == 0:
                                    emit_tail_denom(tci)
                                else:
                                    emit_tail_end(tci)
                            emit_unit()
                            # catch up if behind the 1-unit-per-step schedule
                            # (units arrive ~4 steps into each block)
                            while units_done < min(max(0, k - 36),
                                                   units_ready):
                                emit_unit()
                            if k == 48 and b + 1 < B:
                                # prefetch the next batch's first half-chunk
                                # now, ahead of the drain's DMA burst, so
                                # ph1(b+1) starts without waiting on queues
                                xpt = xprep.tile([128, 16, CH1], bf16,
                                                 tag="xpre", name=f"xpre{b+1}")
                                nc.sync.dma_start(
                                    xpt[:], xh_d[(b + 1) * NCH1, :, 0:16, :])
                                xn_pre[b + 1] = xpt
                        k = len(stream)
                        while fifo:
                            cci, cg, ceS = fifo.pop(0)
                            consume(k, cci, cg, ceS)
                            k += 1
                        # leftover ready units: dependency-free PE cover
                        # while the last chunk's tail chain (DVE add ->
                        # denom matmul -> recip -> mul) resolves
                        for _ in range(min(6, units_ready - units_done)):
                            emit_unit()
                        while pend_tail:
                            _, kind, tci = pend_tail.pop(0)
                            if kind == 0:
                                emit_tail_denom(tci)
                            else:
                                emit_tail_end(tci)
                        while units_done < len(unit_list):
                            emit_unit(drain=True)

    nc.compile()
    return nc


def _prep_inputs(x, freqs_cos, freqs_sin, wq, wk, wv, wo):
    x = np.asarray(x, np.float32)
    c = np.asarray(freqs_cos, np.float32)
    s = np.asarray(freqs_sin, np.float32)
    wq = np.asarray(wq, np.float32)
    wk = np.asarray(wk, np.float32)
    wv = np.asarray(wv, np.float32)
    wo = np.asarray(wo, np.float32)
    bf = ml_dtypes.bfloat16

    xT = x.reshape(T, E).T.astype(bf)
    xh = np.ascontiguousarray(
        xT.reshape(NKB, 128, T // CH1, CH1).transpose(2, 1, 0, 3))

    def fold(w):
        wr = w.reshape(H, D // 2, 2, E)
        w0, w1 = wr[:, :, 0], wr[:, :, 1]
        r0 = c[:, :, None] * w0 - s[:, :, None] * w1
        r1 = s[:, :, None] * w0 + c[:, :, None] * w1
        return np.stack([r0, r1], axis=2).reshape(E, E)

    wq_r = fold(wq) * np.float32(D ** -0.5)
    wk_r = fold(wk)

    in_maps = []
    for cix in range(NCORES):
        sl = slice(cix * W, (cix + 1) * W)
        qk = np.concatenate([wq_r[sl].T, wk_r[sl].T], axis=1)   # [E, 2W]
        wqkh = np.ascontiguousarray(
            qk.astype(bf).reshape(NKB, 128, 2 * W).transpose(1, 0, 2))
        wvh = np.ascontiguousarray(
            wv[sl].T.astype(bf).reshape(NKB, 128, W).transpose(1, 0, 2))
        woh = np.ascontiguousarray(
            wo[:, sl].T.astype(bf).reshape(HL, 128, E).transpose(1, 0, 2))
        in_maps.append({"xh": xh, "wqk": wqkh, "wv": wvh, "wo": woh})
    return in_maps


def run(x, freqs_cos, freqs_sin, wq, wk, wv, wo, trace=False, tmpdir=None):
    from concourse.bass_utils import run_bass_kernel_spmd

    if "nc" not in _CACHE:
        _CACHE["nc"] = _build_nc()
    nc = _CACHE["nc"]
    in_maps = _prep_inputs(x, freqs_cos, freqs_sin, wq, wk, wv, wo)
    res = run_bass_kernel_spmd(
        nc, in_maps, list(range(NCORES)), trace=trace, tmpdir=tmpdir
    )
    y = np.asarray(res.results[0]["y"], np.float32)
    for r in res.results[1:]:
        y = y + np.asarray(r["y"], np.float32)
    # [b, nE, m, row, col] -> [b, m*128+row, nE*512+col]
    y = y.transpose(0, 2, 3, 1, 4).reshape(B, S, E)
    return y, res


def kernel(x, start_pos=0, freqs_cos=None, freqs_sin=None,
           wq=None, wk=None, wv=None, wo=None):
    y, _ = run(x, freqs_cos, freqs_sin, wq, wk, wv, wo)
    return y

